# revision 1
# baseline (speedup 1.0000x reference)
"""GCN+GAT message-passing network on 8 Trainium2 NeuronCores.

Strategy (dst-sharded edge-parallel):
  - Nodes striped round-robin across 8 cores (node n -> core n%8, slot n//8).
  - Per layer, a node table T_l (bf16) holds the per-node rows needed by the
    aggregation: [features | alpha_src | alpha_dst].  Each core aggregates its
    own 49 tiles of 128 dst nodes: per tile it dma_gathers the src rows of its
    incoming edges, builds a one-hot (dst-local x edge) mask with iota +
    is_equal, and lets the TensorEngine do the segment-sums as mask.T @ rhs.
    GAT softmax uses a global upper bound M=30 instead of per-segment max
    (mathematically identical after normalization), normalization happens
    after aggregation (divide by the aggregated exp-sum column).
  - Each core then produces its slice of the next layer's table with dense
    matmuls (weights pre-fused with the attention vectors host-side), and the
    slices are AllGathered so every core again has the full table.
  - dma_gather has int16 indices, so each table is gathered as two halves.
"""

import numpy as np
import ml_dtypes

import concourse.bass as bass
import concourse.bacc as bacc
import concourse.mybir as mybir
import concourse.tile as tile
from concourse.bass_utils import run_bass_kernel_spmd
from concourse.masks import make_identity

BF = ml_dtypes.bfloat16
P = 128
NCORES = 8
N = 50000
E = 800000
NPC = 6272                # nodes per core (49 tiles)
NP = NPC * NCORES         # padded node count 50176
TPC = NPC // P            # 49 tiles per core
HALF = NP // 2            # 25088, int16-safe table half
IN, HC, HEADS, OUT = 128, 64, 4, 32
NEG = 0.2
MBOUND = 30.0             # global upper bound for segment-softmax max-shift

# table geometry: (row stride elems, feat width, alpha_s col, alpha_d col)
ST_GCN = 128              # T1, T2: [feat 64 | pad]
ST_GAT = 384              # T3, T4: [feat 256 | as 4 | ad 4 | pad]
ST_L5 = 128               # T5:     [feat 32 | as 1 | ad 1 | pad]

_PLAN_CACHE = {}


def _build_plan(edge_index):
    """Host-side sharding: per-core gather indices + dst-local tables.

    Chunk layout groups GB consecutive tiles per dma_gather call pair:
    per group: [t0-lo chunks | t1-lo ... | t0-hi | t1-hi ...].
    """
    GB = 2
    src = np.concatenate([edge_index[0], np.arange(N, dtype=np.int64)])
    dst = np.concatenate([edge_index[1], np.arange(N, dtype=np.int64)])
    deg = np.bincount(dst, minlength=N).astype(np.float32)

    nodes = np.arange(N, dtype=np.int64)
    gsid_of = (nodes % NCORES) * NPC + nodes // NCORES   # storage id per node

    gsrc = gsid_of[src]
    gdst = gsid_of[dst]
    hi = (gsrc >= HALF).astype(np.int64)
    tileg = gdst // P                                    # global tile 0..391
    key = tileg * 2 + hi
    order = np.argsort(key, kind="stable")
    sgsrc = gsrc[order]
    sdloc = (gdst[order] % P).astype(np.int64)
    counts = np.bincount(key, minlength=392 * 2).reshape(NCORES, TPC, 2)
    starts = np.zeros(392 * 2 + 1, np.int64)
    np.cumsum(np.bincount(key, minlength=392 * 2), out=starts[1:])

    # uniform-per-position chunk counts (same program on all cores)
    Glo = np.maximum(1, (counts[:, :, 0].max(axis=0) + P - 1) // P)  # [49]
    Ghi = np.maximum(1, (counts[:, :, 1].max(axis=0) + P - 1) // P)

    groups = [list(range(g, min(g + GB, TPC))) for g in range(0, TPC, GB)]
    lo_off = np.zeros(TPC, np.int64)
    hi_off = np.zeros(TPC, np.int64)
    gch_off = []
    gglo = []
    gghi = []
    ch = 0
    for tl in groups:
        gch_off.append(ch)
        glo = int(sum(Glo[t] for t in tl))
        ghi = int(sum(Ghi[t] for t in tl))
        gglo.append(glo)
        gghi.append(ghi)
        o = ch
        for t in tl:
            lo_off[t] = o
            o += Glo[t]
        for t in tl:
            hi_off[t] = o
            o += Ghi[t]
        ch = o
    CH = ch

    idx16 = np.zeros((NCORES, 16, CH * 8), np.int16)
    dstloc = np.full((NCORES, P, CH), -1.0, np.float32)
    for c in range(NCORES):
        for t in range(TPC):
            for h, base in ((0, lo_off[t]), (1, hi_off[t])):
                k = ((c * TPC + t) * 2 + h)
                a, b = int(starts[k]), int(starts[k + 1])
                n = b - a
                if n == 0:
                    continue
                gs = sgsrc[a:b] - (HALF if h else 0)
                dl = sdloc[a:b]
                s2 = np.arange(n)
                idx16[c, s2 % 16, int(base) * 8 + s2 // 16] = gs.astype(np.int16)
                dstloc[c, s2 % P, int(base) + s2 // P] = dl
    deg_g = np.ones(NP, np.float32)
    deg_g[gsid_of] = deg
    degT = np.ascontiguousarray(deg_g.reshape(NP // P, P).T)  # [128, 392]

    gchm = max(gglo[i] + gghi[i] for i in range(len(groups)))
    return dict(
        gsid_of=gsid_of,
        degT=degT,
        idx16=np.tile(idx16, (1, 8, 1)),
        dstloc_bf=dstloc.astype(BF),
        Glo=[int(v) for v in Glo],
        Ghi=[int(v) for v in Ghi],
        lo_off=[int(v) for v in lo_off],
        hi_off=[int(v) for v in hi_off],
        groups=groups,
        gch_off=gch_off,
        gglo=gglo,
        gghi=gghi,
        CH=CH,
        CHMAX=int((Glo + Ghi).max()),
        GCHM=gchm,
    )


def _fuse_gat_weights(W, a_s, a_d, heads, ch):
    Wr = np.asarray(W, np.float32).reshape(W.shape[0], heads, ch)
    Bs = np.einsum("chk,hk->ch", Wr, np.asarray(a_s, np.float32))
    Bd = np.einsum("chk,hk->ch", Wr, np.asarray(a_d, np.float32))
    return np.concatenate([np.asarray(W, np.float32), Bs, Bd], axis=1)


def _build_nc(plan):
    fp32 = mybir.dt.float32
    bf16 = mybir.dt.bfloat16
    CH, CHM, GCHM = plan["CH"], plan["CHMAX"], plan["GCHM"]
    Glo, Ghi = plan["Glo"], plan["Ghi"]
    lo_off, hi_off = plan["lo_off"], plan["hi_off"]
    groups, gch_off = plan["groups"], plan["gch_off"]
    gglo, gghi = plan["gglo"], plan["gghi"]

    nc = bacc.Bacc("TRN2", target_bir_lowering=False, debug=False,
                   num_devices=NCORES)

    xT_in = nc.dram_tensor("xT", [P, NPC], bf16, kind="ExternalInput")
    degO_in = nc.dram_tensor("degO", [P, TPC], fp32, kind="ExternalInput")
    idx_in = nc.dram_tensor("idx16", [P, CH * 8], mybir.dt.int16, kind="ExternalInput")
    dl_in = nc.dram_tensor("dstloc", [P, CH], bf16, kind="ExternalInput")
    w1_in = nc.dram_tensor("W1", [IN, HC], bf16, kind="ExternalInput")
    w2_in = nc.dram_tensor("W2", [HC, HC], bf16, kind="ExternalInput")
    w3_in = nc.dram_tensor("W3e", [HC, 264], bf16, kind="ExternalInput")
    w4_in = nc.dram_tensor("W4p", [P, 2 * 264], bf16, kind="ExternalInput")
    w5_in = nc.dram_tensor("W5p", [P, 2 * 34], bf16, kind="ExternalInput")
    b_in = {}
    for name, n in [("b1", HC), ("b2", HC), ("b3", 256), ("b4", 256), ("b5", OUT)]:
        b_in[name] = nc.dram_tensor(name, [1, n], fp32, kind="ExternalInput")
    out_ext = nc.dram_tensor("out", [NPC, OUT], fp32, kind="ExternalOutput")

    T1s = nc.dram_tensor("T1s", [NPC, ST_GCN], bf16)
    T1 = nc.dram_tensor("T1", [NP, ST_GCN], bf16, addr_space="Shared")
    T2s = nc.dram_tensor("T2s", [NPC, ST_GCN], bf16)
    T2 = nc.dram_tensor("T2", [NP, ST_GCN], bf16, addr_space="Shared")
    T3s = nc.dram_tensor("T3s", [NPC, ST_GAT], bf16)
    T3 = nc.dram_tensor("T3", [NP, ST_GAT], bf16, addr_space="Shared")
    T4s = nc.dram_tensor("T4s", [NPC, ST_GAT], bf16)
    T4 = nc.dram_tensor("T4", [NP, ST_GAT], bf16, addr_space="Shared")
    T5s = nc.dram_tensor("T5s", [NPC, ST_L5], bf16)
    T5 = nc.dram_tensor("T5", [NP, ST_L5], bf16, addr_space="Shared")

    RG = [list(range(NCORES))]

    with tile.TileContext(nc) as tc:
        with (
            tc.tile_pool(name="const", bufs=1) as cp,
            tc.tile_pool(name="work", bufs=2) as wp,
            tc.tile_pool(name="small", bufs=3) as sp,
            tc.tile_pool(name="psum", bufs=2, space="PSUM") as pp,
        ):
            # ---- constants ----
            idx_sb = cp.tile([P, CH * 8], mybir.dt.int16)
            nc.sync.dma_start(out=idx_sb[:], in_=idx_in[:, :])
            dma_sem = nc.alloc_semaphore("gather_dma")
            gstate = {"count": 0, "last_trig": None}
            dl_sb = cp.tile([P, CH], bf16)
            nc.sync.dma_start(out=dl_sb[:], in_=dl_in[:, :])
            w1_sb = cp.tile([IN, HC], bf16)
            nc.sync.dma_start(out=w1_sb[:], in_=w1_in[:, :])
            w2_sb = cp.tile([HC, HC], bf16)
            nc.sync.dma_start(out=w2_sb[:], in_=w2_in[:, :])
            w3_sb = cp.tile([HC, 264], bf16)
            nc.sync.dma_start(out=w3_sb[:], in_=w3_in[:, :])
            w4_sb = cp.tile([P, 2, 264], bf16)
            nc.sync.dma_start(out=w4_sb[:], in_=w4_in[:, :].rearrange("p (k n) -> p k n", k=2))
            w5_sb = cp.tile([P, 2, 34], bf16)
            nc.sync.dma_start(out=w5_sb[:], in_=w5_in[:, :].rearrange("p (k n) -> p k n", k=2))

            b_sb = {}
            for name, n in [("b1", HC), ("b2", HC), ("b3", 256), ("b4", 256), ("b5", OUT)]:
                stage = sp.tile([1, n], fp32, tag="bstage")
                nc.sync.dma_start(out=stage[:], in_=b_in[name][:, :])
                b_sb[name] = cp.tile([P, n], fp32, tag=f"bias_{name}",
                                     name=f"bias_{name}")
                nc.gpsimd.partition_broadcast(b_sb[name][:], stage[:])

            degO = cp.tile([P, TPC], fp32)
            nc.sync.dma_start(out=degO[:], in_=degO_in[:, :])
            nc.scalar.activation(degO[:], degO[:], mybir.ActivationFunctionType.Sqrt)
            nc.vector.reciprocal(degO[:], degO[:])

            mb_sb = cp.tile([P, 1], fp32, name="mbound")
            nc.gpsimd.memset(mb_sb[:], -MBOUND)

            iota_i = cp.tile([P, P], mybir.dt.int32)
            nc.gpsimd.iota(iota_i[:], pattern=[[1, P]], base=0, channel_multiplier=0)
            iota_bf = cp.tile([P, P], bf16)
            nc.vector.tensor_copy(iota_bf[:], iota_i[:])
            ident = cp.tile([P, P], bf16)
            make_identity(nc, ident[:])

            ad3 = cp.tile([P, TPC * HEADS], fp32, tag="ad3")
            ad4 = cp.tile([P, TPC * HEADS], fp32, tag="ad4")
            ad5 = cp.tile([P, TPC], fp32, tag="ad5")

            AF = mybir.ActivationFunctionType
            ALU = mybir.AluOpType

            def elu_to_bf16(pre, n, tag):
                """H = max(pre, exp(min(pre,0)) - 1), bf16 output."""
                mn = sp.tile([P, n], fp32, tag=f"elu_mn_{tag}", name="elu_mn")
                nc.vector.tensor_scalar_min(mn[:], pre[:], 0.0)
                nc.scalar.activation(mn[:], mn[:], AF.Exp)
                nc.vector.tensor_scalar_add(mn[:], mn[:], -1.0)
                h = sp.tile([P, n], bf16, tag=f"elu_h_{tag}", name="elu_h")
                nc.vector.tensor_tensor(out=h[:], in0=pre[:], in1=mn[:], op=ALU.max)
                return h

            def grouped_gather(gi, hgt, elem, tbl, tok_step):
                """Gather all chunks of tile-group gi into hgt; completion token
                carries the DMA semaphore wait so consumer RAW edges gate on
                actual data arrival."""
                glo, ghi = gglo[gi], gghi[gi]
                gcht = glo + ghi
                for brel, G in ((0, glo), (glo, ghi)):
                    cbase = (gch_off[gi] + brel) * 8
                    src_ap = tbl[0:HALF, :] if brel == 0 else tbl[HALF:NP, :]
                    prep = nc.gpsimd.dma_gather(
                        out_ap=hgt[:, brel:brel + G, :],
                        in_ap=src_ap,
                        idxs_ap=idx_sb[0:16, cbase:cbase + G * 8],
                        num_idxs=G * P,
                        num_idxs_reg=G * P,
                        elem_size=elem,
                        prepare_only=True,
                        sem=dma_sem,
                        single_packet=False,
                    )
                    if gstate["last_trig"] is not None:
                        bass._add_dep_helper(prep.ins, gstate["last_trig"].ins,
                                             sync=False, reason="ring order")
                    gstate["count"] += 16
                trig = nc.gpsimd.trigger_dma(count=None)
                gstate["last_trig"] = trig
                tok_ap = hgt[:, 0:gcht, :].rearrange(
                    "p g (a b) -> p g a b", b=tok_step)[:, :, :, 0:1]
                tok = nc.gpsimd.tensor_copy(tok_ap, tok_ap)
                tok._wait_ge(dma_sem, gstate["count"])
                bass._add_dep_helper(tok.ins, trig.ins, sync=False,
                                     reason="token after trigger")

            def group_mask(gi):
                gcht = gglo[gi] + gghi[gi]
                m = wp.tile([P, GCHM, P], bf16, tag="mask", name="maskg")
                nc.vector.tensor_tensor(
                    out=m[:, 0:gcht, :],
                    in0=iota_bf[:].rearrange("p (g d) -> p g d", g=1).to_broadcast([P, gcht, P]),
                    in1=dl_sb[:, gch_off[gi]:gch_off[gi] + gcht]
                        .rearrange("p (g d) -> p g d", d=1).to_broadcast([P, gcht, P]),
                    op=ALU.is_equal,
                )
                return m

            def tile_ranges(gi, t):
                base = gch_off[gi]
                return ((lo_off[t] - base, Glo[t]), (hi_off[t] - base, Ghi[t]))

            def agg_matmuls(gi, t, maskg, hgt, width):
                acc = pp.tile([P, 264], fp32, tag="agg", space="PSUM", name="agg")
                chunks = []
                for off, G in tile_ranges(gi, t):
                    chunks += list(range(off, off + G))
                for i, g in enumerate(chunks):
                    nc.tensor.matmul(
                        out=acc[:, 0:width],
                        lhsT=maskg[:, g, :],
                        rhs=hgt[:, g, 0:width],
                        start=(i == 0),
                        stop=(i == len(chunks) - 1),
                    )
                return acc

            # ================= L0: T1 = dinv * (x @ W1), own slice ==========
            SUP = 4
            for j0 in range(0, TPC, SUP):
                ks = min(SUP, TPC - j0)
                xsl = wp.tile([P, SUP * P], bf16, tag="xsl", name="xsl")
                nc.sync.dma_start(out=xsl[:, 0:ks * P],
                                  in_=xT_in[:, j0 * P:(j0 + ks) * P])
                t1b = wp.tile([P, SUP, HC], bf16, tag="t1b", name="t1b")
                for k in range(ks):
                    ps = pp.tile([P, 264], fp32, tag="agg", space="PSUM", name="psl0")
                    nc.tensor.matmul(out=ps[:, 0:HC], lhsT=xsl[:, k * P:(k + 1) * P],
                                     rhs=w1_sb[:], start=True, stop=True)
                    nc.scalar.activation(t1b[:, k, :], ps[:, 0:HC], AF.Copy,
                                         scale=degO[:, j0 + k:j0 + k + 1])
                nc.sync.dma_start(
                    out=T1s[j0 * P:(j0 + ks) * P, 0:HC]
                        .rearrange("(a p) c -> p a c", p=P),
                    in_=t1b[:, 0:ks, :],
                )
            nc.gpsimd.collective_compute(
                "AllGather", mybir.AluOpType.bypass, replica_groups=RG,
                ins=[T1s[:, :]], outs=[T1[:, :]],
            )

            # ================= GCN layers ===================================
            def gcn_layer(tbl, bias, wnext, wnext_width, slice_out, slice_width,
                          scale_next, ad_save, lname):
                for gi, tl in enumerate(groups):
                    hg = wp.tile([P, GCHM, ST_GCN], bf16, tag="hg_gcn", name="hg")
                    grouped_gather(gi, hg, ST_GCN, tbl, 64)
                    maskg = group_mask(gi)
                    for t in tl:
                        acc = agg_matmuls(gi, t, maskg, hg, HC)
                        pre = sp.tile([P, HC], fp32, tag="pre64", name="pre")
                        nc.scalar.activation(pre[:], acc[:, 0:HC], AF.Copy,
                                             scale=degO[:, t:t + 1])
                        nc.vector.tensor_tensor(out=pre[:], in0=pre[:],
                                                in1=b_sb[bias][:], op=ALU.add)
                        h = elu_to_bf16(pre, HC, "64")
                        tp = pp.tile([P, P], bf16, tag="tpose", space="PSUM", name="tp")
                        nc.tensor.transpose(tp[0:HC, :], h[:], ident[:])
                        ht = sp.tile([HC, P], bf16, tag="ht64", name="ht")
                        nc.vector.tensor_copy(ht[:], tp[0:HC, :])
                        wout = pp.tile([P, 264], fp32, tag="wout", space="PSUM", name="wout")
                        nc.tensor.matmul(out=wout[:, 0:wnext_width], lhsT=ht[:],
                                         rhs=wnext[:], start=True, stop=True)
                        tt = sp.tile([P, wnext_width], bf16, tag=f"ttile_{lname}",
                                     name="ttile")
                        if scale_next is not None:
                            nc.scalar.activation(tt[:], wout[:, 0:wnext_width], AF.Copy,
                                                 scale=scale_next[:, t:t + 1])
                        else:
                            nc.vector.tensor_copy(tt[:], wout[:, 0:wnext_width])
                        if ad_save is not None:
                            nc.vector.tensor_copy(ad_save[:, t * HEADS:(t + 1) * HEADS],
                                                  wout[:, 260:264])
                        nc.sync.dma_start(
                            out=slice_out[t * P:(t + 1) * P, 0:slice_width],
                            in_=tt[:],
                        )

            gcn_layer(T1, "b1", w2_sb[:], HC, T2s, HC, degO, None, "t2")
            nc.gpsimd.collective_compute(
                "AllGather", mybir.AluOpType.bypass, replica_groups=RG,
                ins=[T2s[:, :]], outs=[T2[:, :]],
            )
            gcn_layer(T2, "b2", w3_sb[:], 264, T3s, 264, None, ad3, "t3")
            nc.gpsimd.collective_compute(
                "AllGather", mybir.AluOpType.bypass, replica_groups=RG,
                ins=[T3s[:, :]], outs=[T3[:, :]],
            )

            # ================= GAT layers ===================================
            def gat_attn_inplace(gi, t, hg, ad_prev, nheads, fw, hc):
                """Per-tile, per-range: hg[:, :, fw:fw+nheads] <- exp(lrelu(
                as+ad)-M); hg[:, :, 0:fw] *= that (broadcast per head)."""
                for off, G in tile_ranges(gi, t):
                    e = sp.tile([P, CHM, HEADS], fp32, tag="ebuf", name="ebuf")
                    nc.vector.tensor_tensor(
                        out=e[:, 0:G, 0:nheads],
                        in0=hg[:, off:off + G, fw:fw + nheads],
                        in1=ad_prev[:, t * nheads:(t + 1) * nheads]
                            .rearrange("p (g h) -> p g h", g=1)
                            .to_broadcast([P, G, nheads]),
                        op=ALU.add,
                    )
                    e2 = sp.tile([P, CHM, HEADS], fp32, tag="ebuf2", name="ebuf2")
                    nc.vector.tensor_scalar_mul(e2[:, 0:G, 0:nheads],
                                                e[:, 0:G, 0:nheads], NEG)
                    nc.vector.tensor_tensor(out=e2[:, 0:G, 0:nheads],
                                            in0=e2[:, 0:G, 0:nheads],
                                            in1=e[:, 0:G, 0:nheads], op=ALU.max)
                    nc.scalar.activation(hg[:, off:off + G, fw:fw + nheads],
                                         e2[:, 0:G, 0:nheads],
                                         AF.Exp, bias=mb_sb[:, 0:1])
                    nc.vector.tensor_tensor(
                        out=hg[:, off:off + G, 0:fw].rearrange(
                            "p g (h z) -> p g h z", z=hc),
                        in0=hg[:, off:off + G, 0:fw].rearrange(
                            "p g (h z) -> p g h z", z=hc),
                        in1=hg[:, off:off + G, fw:fw + nheads]
                            .rearrange("p g (h z) -> p g h z", z=1)
                            .to_broadcast([P, G, nheads, hc]),
                        op=ALU.mult,
                    )

            def gat_layer(tbl, ad_prev, bias, wnext_sb, wnext_width, slice_out,
                          slice_width, ad_save, ad_width, lname):
                for gi, tl in enumerate(groups):
                    hg = wp.tile([P, GCHM, ST_GAT], bf16, tag="hg_gat", name="hg")
                    grouped_gather(gi, hg, ST_GAT, tbl, 128)
                    maskg = group_mask(gi)
                    for t in tl:
                        gat_attn_inplace(gi, t, hg, ad_prev, HEADS, 256, HC)
                        acc = agg_matmuls(gi, t, maskg, hg, 260)
                        rs = sp.tile([P, HEADS], fp32, tag="rs4", name="rs")
                        nc.vector.tensor_scalar_max(rs[:], acc[:, 256:260], 1e-30)
                        nc.vector.reciprocal(rs[:], rs[:])
                        pre = sp.tile([P, 256], fp32, tag="pre256", name="pre")
                        nc.vector.tensor_tensor(
                            out=pre[:].rearrange("p (h z) -> p h z", z=HC),
                            in0=acc[:, 0:256].rearrange("p (h z) -> p h z", z=HC),
                            in1=rs[:].rearrange("p (h z) -> p h z", z=1)
                                .to_broadcast([P, HEADS, HC]),
                            op=ALU.mult,
                        )
                        nc.vector.tensor_tensor(out=pre[:], in0=pre[:],
                                                in1=b_sb[bias][:], op=ALU.add)
                        h = elu_to_bf16(pre, 256, "256")
                        ht = sp.tile([P, 2, P], bf16, tag="ht256", name="ht")
                        for k in range(2):
                            tp = pp.tile([P, P], bf16, tag="tpose", space="PSUM",
                                         name="tp")
                            nc.tensor.transpose(tp[:], h[:, k * P:(k + 1) * P], ident[:])
                            nc.vector.tensor_copy(ht[:, k, :], tp[:])
                        wout = pp.tile([P, 264], fp32, tag="wout", space="PSUM",
                                       name="wout")
                        for k in range(2):
                            nc.tensor.matmul(out=wout[:, 0:wnext_width],
                                             lhsT=ht[:, k, :],
                                             rhs=wnext_sb[:, k, :wnext_width],
                                             start=(k == 0), stop=(k == 1))
                        tt = sp.tile([P, wnext_width], bf16, tag=f"ttile_{lname}",
                                     name="ttile")
                        nc.vector.tensor_copy(tt[:], wout[:, 0:wnext_width])
                        nc.vector.tensor_copy(
                            ad_save[:, t * ad_width:(t + 1) * ad_width],
                            wout[:, wnext_width - ad_width:wnext_width])
                        nc.sync.dma_start(
                            out=slice_out[t * P:(t + 1) * P, 0:slice_width],
                            in_=tt[:],
                        )

            gat_layer(T3, ad3, "b3", w4_sb, 264, T4s, 264, ad4, HEADS, "t4")
            nc.gpsimd.collective_compute(
                "AllGather", mybir.AluOpType.bypass, replica_groups=RG,
                ins=[T4s[:, :]], outs=[T4[:, :]],
            )
            gat_layer(T4, ad4, "b4", w5_sb, 34, T5s, 34, ad5, 1, "t5")
            nc.gpsimd.collective_compute(
                "AllGather", mybir.AluOpType.bypass, replica_groups=RG,
                ins=[T5s[:, :]], outs=[T5[:, :]],
            )

            # ================= L5: single-head GAT + log_softmax ============
            for gi, tl in enumerate(groups):
                hg = wp.tile([P, GCHM, ST_L5], bf16, tag="hg_gcn", name="hg")
                grouped_gather(gi, hg, ST_L5, T5, 32)
                maskg = group_mask(gi)
                for t in tl:
                    gat_attn_inplace(gi, t, hg, ad5, 1, 32, 32)
                    acc = agg_matmuls(gi, t, maskg, hg, 33)
                    rs = sp.tile([P, 1], fp32, tag="rs1", name="rs")
                    nc.vector.tensor_scalar_max(rs[:], acc[:, 32:33], 1e-30)
                    nc.vector.reciprocal(rs[:], rs[:])
                    logits = sp.tile([P, OUT], fp32, tag="logits", name="logits")
                    nc.scalar.activation(logits[:], acc[:, 0:OUT], AF.Copy,
                                         scale=rs[:, 0:1])
                    nc.vector.tensor_tensor(out=logits[:], in0=logits[:],
                                            in1=b_sb["b5"][:], op=ALU.add)
                    mneg = sp.tile([P, 1], fp32, tag="mneg", name="mneg")
                    nc.vector.tensor_reduce(out=mneg[:], in_=logits[:],
                                            axis=mybir.AxisListType.X,
                                            op=ALU.max, negate=True)
                    ex2 = sp.tile([P, OUT], fp32, tag="ex2", name="ex2")
                    ssum = sp.tile([P, 1], fp32, tag="ssum", name="ssum")
                    nc.scalar.activation(ex2[:], logits[:], AF.Exp, bias=mneg[:, 0:1],
                                         accum_out=ssum[:])
                    lg = sp.tile([P, 1], fp32, tag="lg", name="lg")
                    nc.scalar.activation(lg[:], ssum[:], AF.Ln)
                    cc2 = sp.tile([P, 1], fp32, tag="cc", name="cc2")
                    nc.vector.tensor_tensor(out=cc2[:], in0=mneg[:], in1=lg[:],
                                            op=ALU.subtract)
                    outt = sp.tile([P, OUT], fp32, tag="outt", name="outt")
                    nc.scalar.activation(outt[:], logits[:], AF.Identity,
                                         bias=cc2[:, 0:1])
                    nc.sync.dma_start(out=out_ext[t * P:(t + 1) * P, :], in_=outt[:])

    nc.finalize()
    return nc


def _prepare(inputs):
    edge_index = np.asarray(inputs["edge_index"])
    plan = _build_plan(edge_index)

    x = np.asarray(inputs["x"], np.float32)
    x_g = np.zeros((NP, IN), np.float32)
    x_g[plan["gsid_of"]] = x
    xT = np.ascontiguousarray(x_g.T).astype(BF)

    W3e = _fuse_gat_weights(inputs["gat1_W"], inputs["gat1_as"], inputs["gat1_ad"], HEADS, HC)
    W4e = _fuse_gat_weights(inputs["gat2_W"], inputs["gat2_as"], inputs["gat2_ad"], HEADS, HC)
    W5e = _fuse_gat_weights(inputs["gat3_W"], inputs["gat3_as"], inputs["gat3_ad"], 1, OUT)
    W4p = np.ascontiguousarray(
        W4e.reshape(2, P, 264).transpose(1, 0, 2).reshape(P, 2 * 264))
    W5p = np.ascontiguousarray(
        W5e.reshape(2, P, 34).transpose(1, 0, 2).reshape(P, 2 * 34))

    common = {
        "xT": xT,
        "degT": plan["degT"],
        "W1": np.asarray(inputs["gcn1_W"], np.float32).astype(BF),
        "W2": np.asarray(inputs["gcn2_W"], np.float32).astype(BF),
        "W3e": W3e.astype(BF),
        "W4p": W4p.astype(BF),
        "W5p": W5p.astype(BF),
        "b1": np.asarray(inputs["gcn1_b"], np.float32).reshape(1, -1),
        "b2": np.asarray(inputs["gcn2_b"], np.float32).reshape(1, -1),
        "b3": np.asarray(inputs["gat1_b"], np.float32).reshape(1, -1),
        "b4": np.asarray(inputs["gat2_b"], np.float32).reshape(1, -1),
        "b5": np.asarray(inputs["gat3_b"], np.float32).reshape(1, -1),
    }
    in_maps = []
    for c in range(NCORES):
        m = dict(common)
        m["degO"] = np.ascontiguousarray(plan["degT"][:, c * TPC:(c + 1) * TPC])
        m["idx16"] = np.ascontiguousarray(plan["idx16"][c])
        m["dstloc"] = np.ascontiguousarray(plan["dstloc_bf"][c])
        in_maps.append(m)
    return plan, in_maps


def kernel(**inputs) -> np.ndarray:
    plan, in_maps = _prepare(inputs)
    nc = _build_nc(plan)
    res = run_bass_kernel_spmd(nc, in_maps, core_ids=list(range(NCORES)))
    out_g = np.concatenate([res.results[c]["out"] for c in range(NCORES)], axis=0)
    return np.ascontiguousarray(out_g[plan["gsid_of"]]).astype(np.float32)


if __name__ == "__main__":
    rng = np.random.default_rng(0)
    data = dict(np.load("/tmp/inputs.npz"))
    out = kernel(**data)
    exp = np.load("/tmp/expected.npy")
    d = np.abs(out - exp)
    print("max abs:", d.max(), "rel:", d.max() / np.abs(exp).max())



# revision 11
# speedup vs baseline: 1.1424x; 1.1424x over previous
"""GCN+GAT message-passing network on 8 Trainium2 NeuronCores.

Strategy (dst-sharded edge-parallel):
  - Nodes striped round-robin across 8 cores (node n -> core n%8, slot n//8).
  - Per layer, a node table T_l (bf16) holds the per-node rows needed by the
    aggregation: [features | alpha_src | alpha_dst].  Each core aggregates its
    own 49 tiles of 128 dst nodes: per tile it dma_gathers the src rows of its
    incoming edges, builds a one-hot (dst-local x edge) mask with iota +
    is_equal, and lets the TensorEngine do the segment-sums as mask.T @ rhs.
    GAT softmax uses a global upper bound M=30 instead of per-segment max
    (mathematically identical after normalization), normalization happens
    after aggregation (divide by the aggregated exp-sum column).
  - Each core then produces its slice of the next layer's table with dense
    matmuls (weights pre-fused with the attention vectors host-side), and the
    slices are AllGathered so every core again has the full table.
  - dma_gather has int16 indices, so each table is gathered as two halves.
"""

import numpy as np
import ml_dtypes

import concourse.bass as bass
import concourse.bacc as bacc
import concourse.mybir as mybir
import concourse.tile as tile
from concourse.bass_utils import run_bass_kernel_spmd
from concourse.masks import make_identity

BF = ml_dtypes.bfloat16
P = 128
NCORES = 8
N = 50000
E = 800000
NPC = 6272                # nodes per core (49 tiles)
NP = NPC * NCORES         # padded node count 50176
TPC = NPC // P            # 49 tiles per core
HALF = NP // 2            # 25088, int16-safe table half
IN, HC, HEADS, OUT = 128, 64, 4, 32
NEG = 0.2
MBOUND = 30.0             # global upper bound for segment-softmax max-shift

# table geometry: (row stride elems, feat width, alpha_s col, alpha_d col)
ST_GCN = 128              # T1, T2: [feat 64 | pad]
ST_GAT = 384              # T3, T4: [feat 256 | as 4 | ad 4 | pad]
ST_L5 = 128               # T5:     [feat 32 | as 1 | ad 1 | pad]

_PLAN_CACHE = {}


def _build_plan(edge_index):
    """Host-side sharding: per-core gather indices + dst-local tables.

    Chunk layout groups GB consecutive tiles per dma_gather call pair:
    per group: [t0-lo chunks | t1-lo ... | t0-hi | t1-hi ...].

    Nodes are assigned to (core, tile) buckets by balanced in-degree
    (greedy heap) so every tile's incoming-edge count is near-uniform:
    desc-gen on the Pool engine is ~8ns/edge-slot, so minimizing the
    max-over-cores chunk padding directly cuts the bottleneck.
    """
    import heapq

    GB = 2
    src = np.concatenate([edge_index[0], np.arange(N, dtype=np.int64)])
    dst = np.concatenate([edge_index[1], np.arange(N, dtype=np.int64)])
    deg = np.bincount(dst, minlength=N).astype(np.float32)

    nbuckets = NCORES * TPC
    order_n = np.argsort(-deg, kind="stable")
    counts_b = np.zeros(nbuckets, np.int64)
    bucket = np.empty(N, np.int64)
    slot = np.empty(N, np.int64)
    heap = [(0.0, b) for b in range(nbuckets)]
    heapq.heapify(heap)
    for n in order_n:
        while True:
            load, b = heapq.heappop(heap)
            if counts_b[b] < P:
                break
        bucket[n] = b
        slot[n] = counts_b[b]
        counts_b[b] += 1
        heapq.heappush(heap, (load + float(deg[n]), b))
    gsid_of = (bucket // TPC) * NPC + (bucket % TPC) * P + slot

    gsrc = gsid_of[src]
    gdst = gsid_of[dst]
    hi = (gsrc >= HALF).astype(np.int64)
    tileg = gdst // P                                    # global tile 0..391
    key = tileg * 2 + hi
    order = np.argsort(key, kind="stable")
    sgsrc = gsrc[order]
    sdloc = (gdst[order] % P).astype(np.int64)
    counts = np.bincount(key, minlength=392 * 2).reshape(NCORES, TPC, 2)
    starts = np.zeros(392 * 2 + 1, np.int64)
    np.cumsum(np.bincount(key, minlength=392 * 2), out=starts[1:])

    # uniform-per-position chunk counts (same program on all cores)
    Glo = np.maximum(1, (counts[:, :, 0].max(axis=0) + P - 1) // P)  # [49]
    Ghi = np.maximum(1, (counts[:, :, 1].max(axis=0) + P - 1) // P)

    groups = [list(range(g, min(g + GB, TPC))) for g in range(0, TPC, GB)]
    lo_off = np.zeros(TPC, np.int64)
    hi_off = np.zeros(TPC, np.int64)
    gch_off = []
    gglo = []
    gghi = []
    ch = 0
    for tl in groups:
        gch_off.append(ch)
        glo = int(sum(Glo[t] for t in tl))
        ghi = int(sum(Ghi[t] for t in tl))
        gglo.append(glo)
        gghi.append(ghi)
        o = ch
        for t in tl:
            lo_off[t] = o
            o += Glo[t]
        for t in tl:
            hi_off[t] = o
            o += Ghi[t]
        ch = o
    CH = ch

    ssrc_orig = src[order]          # original src node id per sorted edge
    idx16 = np.zeros((NCORES, 16, CH * 8), np.int16)
    dstloc = np.full((NCORES, P, CH), -1.0, np.float32)
    xgsrc = np.full((NCORES, P, CH), -1, np.int64)   # src node per slot (L1 pregather)
    for c in range(NCORES):
        for t in range(TPC):
            for h, base in ((0, lo_off[t]), (1, hi_off[t])):
                k = ((c * TPC + t) * 2 + h)
                a, b = int(starts[k]), int(starts[k + 1])
                n = b - a
                if n == 0:
                    continue
                gs = sgsrc[a:b] - (HALF if h else 0)
                dl = sdloc[a:b]
                s2 = np.arange(n)
                idx16[c, s2 % 16, int(base) * 8 + s2 // 16] = gs.astype(np.int16)
                dstloc[c, s2 % P, int(base) + s2 // P] = dl
                xgsrc[c, s2 % P, int(base) + s2 // P] = ssrc_orig[a:b]
    deg_g = np.ones(NP, np.float32)
    deg_g[gsid_of] = deg
    degT = np.ascontiguousarray(deg_g.reshape(NP // P, P).T)  # [128, 392]

    gchm = max(gglo[i] + gghi[i] for i in range(len(groups)))
    return dict(
        gsid_of=gsid_of,
        deg=deg,
        degT=degT,
        xgsrc=xgsrc,
        idx16=np.tile(idx16, (1, 8, 1)),
        dstloc_bf=dstloc.astype(BF),
        Glo=[int(v) for v in Glo],
        Ghi=[int(v) for v in Ghi],
        lo_off=[int(v) for v in lo_off],
        hi_off=[int(v) for v in hi_off],
        groups=groups,
        gch_off=gch_off,
        gglo=gglo,
        gghi=gghi,
        CH=CH,
        CHMAX=int((Glo + Ghi).max()),
        GCHM=gchm,
    )


def _fuse_gat_weights(W, a_s, a_d, heads, ch):
    Wr = np.asarray(W, np.float32).reshape(W.shape[0], heads, ch)
    Bs = np.einsum("chk,hk->ch", Wr, np.asarray(a_s, np.float32))
    Bd = np.einsum("chk,hk->ch", Wr, np.asarray(a_d, np.float32))
    return np.concatenate([np.asarray(W, np.float32), Bs, Bd], axis=1)


def _build_nc(plan):
    fp32 = mybir.dt.float32
    bf16 = mybir.dt.bfloat16
    CH, CHM, GCHM = plan["CH"], plan["CHMAX"], plan["GCHM"]
    Glo, Ghi = plan["Glo"], plan["Ghi"]
    lo_off, hi_off = plan["lo_off"], plan["hi_off"]
    groups, gch_off = plan["groups"], plan["gch_off"]
    gglo, gghi = plan["gglo"], plan["gghi"]

    nc = bacc.Bacc("TRN2", target_bir_lowering=False, debug=False,
                   num_devices=NCORES)

    xg_in = nc.dram_tensor("xg", [P, CH * P], bf16, kind="ExternalInput")
    degO_in = nc.dram_tensor("degO", [P, TPC], fp32, kind="ExternalInput")
    idx_in = nc.dram_tensor("idx16", [P, CH * 8], mybir.dt.int16, kind="ExternalInput")
    dl_in = nc.dram_tensor("dstloc", [P, CH], bf16, kind="ExternalInput")
    w1_in = nc.dram_tensor("W1", [IN, HC], bf16, kind="ExternalInput")
    w2_in = nc.dram_tensor("W2", [HC, HC], bf16, kind="ExternalInput")
    w3_in = nc.dram_tensor("W3e", [HC, 264], bf16, kind="ExternalInput")
    w4_in = nc.dram_tensor("W4p", [P, 2 * 264], bf16, kind="ExternalInput")
    w5_in = nc.dram_tensor("W5p", [P, 2 * 34], bf16, kind="ExternalInput")
    b_in = {}
    for name, n in [("b1", HC), ("b2", HC), ("b3", 256), ("b4", 256), ("b5", OUT)]:
        b_in[name] = nc.dram_tensor(name, [1, n], fp32, kind="ExternalInput")
    out_ext = nc.dram_tensor("out", [NPC, OUT], fp32, kind="ExternalOutput")

    T2s = nc.dram_tensor("T2s", [NPC, ST_GCN], bf16)
    T2 = nc.dram_tensor("T2", [NP, ST_GCN], bf16, addr_space="Shared")
    T3s = nc.dram_tensor("T3s", [NPC, ST_GAT], bf16)
    T3 = nc.dram_tensor("T3", [NP, ST_GAT], bf16, addr_space="Shared")
    T4s = nc.dram_tensor("T4s", [NPC, ST_GAT], bf16)
    T4 = nc.dram_tensor("T4", [NP, ST_GAT], bf16, addr_space="Shared")
    T5s = nc.dram_tensor("T5s", [NPC, ST_L5], bf16)
    T5 = nc.dram_tensor("T5", [NP, ST_L5], bf16, addr_space="Shared")

    RG = [list(range(NCORES))]

    with tile.TileContext(nc) as tc:
        with (
            tc.tile_pool(name="const", bufs=1) as cp,
            tc.tile_pool(name="work", bufs=2) as wp,
            tc.tile_pool(name="small", bufs=3) as sp,
            tc.tile_pool(name="psum", bufs=2, space="PSUM") as pp,
        ):
            # ---- constants ----
            idx_sb = cp.tile([P, CH * 8], mybir.dt.int16)
            nc.sync.dma_start(out=idx_sb[:], in_=idx_in[:, :])
            dma_sem = nc.alloc_semaphore("gather_dma")
            gstate = {"count": 0, "last_trig": None}
            dl_sb = cp.tile([P, CH], bf16)
            nc.sync.dma_start(out=dl_sb[:], in_=dl_in[:, :])
            w1_sb = cp.tile([IN, HC], bf16)
            nc.sync.dma_start(out=w1_sb[:], in_=w1_in[:, :])
            w2_sb = cp.tile([HC, HC], bf16)
            nc.sync.dma_start(out=w2_sb[:], in_=w2_in[:, :])
            w3_sb = cp.tile([HC, 264], bf16)
            nc.sync.dma_start(out=w3_sb[:], in_=w3_in[:, :])
            w4_sb = cp.tile([P, 2, 264], bf16)
            nc.sync.dma_start(out=w4_sb[:], in_=w4_in[:, :].rearrange("p (k n) -> p k n", k=2))
            w5_sb = cp.tile([P, 2, 34], bf16)
            nc.sync.dma_start(out=w5_sb[:], in_=w5_in[:, :].rearrange("p (k n) -> p k n", k=2))

            b_sb = {}
            for name, n in [("b1", HC), ("b2", HC), ("b3", 256), ("b4", 256), ("b5", OUT)]:
                stage = sp.tile([1, n], fp32, tag="bstage")
                nc.sync.dma_start(out=stage[:], in_=b_in[name][:, :])
                b_sb[name] = cp.tile([P, n], fp32, tag=f"bias_{name}",
                                     name=f"bias_{name}")
                nc.gpsimd.partition_broadcast(b_sb[name][:], stage[:])

            degO = cp.tile([P, TPC], fp32)
            nc.sync.dma_start(out=degO[:], in_=degO_in[:, :])
            nc.scalar.activation(degO[:], degO[:], mybir.ActivationFunctionType.Sqrt)
            nc.vector.reciprocal(degO[:], degO[:])

            mb_sb = cp.tile([P, 1], fp32, name="mbound")
            nc.gpsimd.memset(mb_sb[:], -MBOUND)

            iota_i = cp.tile([P, P], mybir.dt.int32)
            nc.gpsimd.iota(iota_i[:], pattern=[[1, P]], base=0, channel_multiplier=0)
            iota_bf = cp.tile([P, P], bf16)
            nc.vector.tensor_copy(iota_bf[:], iota_i[:])
            ident = cp.tile([P, P], bf16)
            make_identity(nc, ident[:])

            ad3 = cp.tile([P, TPC * HEADS], fp32, tag="ad3")
            ad4 = cp.tile([P, TPC * HEADS], fp32, tag="ad4")
            ad5 = cp.tile([P, TPC], fp32, tag="ad5")

            AF = mybir.ActivationFunctionType
            ALU = mybir.AluOpType

            def elu_to_bf16(pre, n, tag):
                """H = max(pre, exp(min(pre,0)) - 1), bf16 output."""
                mn = sp.tile([P, n], fp32, tag=f"elu_mn_{tag}", name="elu_mn")
                nc.vector.tensor_scalar_min(mn[:], pre[:], 0.0)
                nc.scalar.activation(mn[:], mn[:], AF.Exp)
                nc.vector.tensor_scalar_add(mn[:], mn[:], -1.0)
                h = sp.tile([P, n], bf16, tag=f"elu_h_{tag}", name="elu_h")
                nc.vector.tensor_tensor(out=h[:], in0=pre[:], in1=mn[:], op=ALU.max)
                return h

            def grouped_gather(gi, hgt, elem, tbl, tok_step):
                """Gather all chunks of tile-group gi into hgt; completion token
                carries the DMA semaphore wait so consumer RAW edges gate on
                actual data arrival."""
                glo, ghi = gglo[gi], gghi[gi]
                gcht = glo + ghi
                for brel, G in ((0, glo), (glo, ghi)):
                    cbase = (gch_off[gi] + brel) * 8
                    src_ap = tbl[0:HALF, :] if brel == 0 else tbl[HALF:NP, :]
                    prep = nc.gpsimd.dma_gather(
                        out_ap=hgt[:, brel:brel + G, :],
                        in_ap=src_ap,
                        idxs_ap=idx_sb[0:16, cbase:cbase + G * 8],
                        num_idxs=G * P,
                        num_idxs_reg=G * P,
                        elem_size=elem,
                        prepare_only=True,
                        sem=dma_sem,
                        single_packet=False,
                    )
                    if gstate["last_trig"] is not None:
                        bass._add_dep_helper(prep.ins, gstate["last_trig"].ins,
                                             sync=False, reason="ring order")
                    gstate["count"] += 16
                trig = nc.gpsimd.trigger_dma(count=None)
                gstate["last_trig"] = trig
                tok_ap = hgt[:, 0:gcht, :].rearrange(
                    "p g (a b) -> p g a b", b=tok_step)[:, :, :, 0:1]
                tok = nc.gpsimd.tensor_copy(tok_ap, tok_ap)
                tok._wait_ge(dma_sem, gstate["count"])
                bass._add_dep_helper(tok.ins, trig.ins, sync=False,
                                     reason="token after trigger")

            def group_mask(gi):
                gcht = gglo[gi] + gghi[gi]
                m = wp.tile([P, GCHM, P], bf16, tag="mask", name="maskg")
                nc.vector.tensor_tensor(
                    out=m[:, 0:gcht, :],
                    in0=iota_bf[:].rearrange("p (g d) -> p g d", g=1).to_broadcast([P, gcht, P]),
                    in1=dl_sb[:, gch_off[gi]:gch_off[gi] + gcht]
                        .rearrange("p (g d) -> p g d", d=1).to_broadcast([P, gcht, P]),
                    op=ALU.is_equal,
                )
                return m

            def tile_ranges(gi, t):
                base = gch_off[gi]
                return ((lo_off[t] - base, Glo[t]), (hi_off[t] - base, Ghi[t]))

            def agg_matmuls(gi, t, maskg, hgt, width):
                acc = pp.tile([P, 264], fp32, tag="agg", space="PSUM", name="agg")
                chunks = []
                for off, G in tile_ranges(gi, t):
                    chunks += list(range(off, off + G))
                for i, g in enumerate(chunks):
                    nc.tensor.matmul(
                        out=acc[:, 0:width],
                        lhsT=maskg[:, g, :],
                        rhs=hgt[:, g, 0:width],
                        start=(i == 0),
                        stop=(i == len(chunks) - 1),
                    )
                return acc

            # ================= L1: host-pregathered x rows ==================
            # xg holds per-edge-slot rows x[src]*dinv[src] (bf16, zero-filled
            # padding), loaded with static-pattern DMA — no Q7 desc-gen.
            # Aggregation uses swapped operands: aggT = xg_chunk.T @ mask
            # gives [feat, dst] in PSUM; then pre = aggT.T @ W1 via a second
            # matmul (lhsT from SBUF), dst-dinv scale applied post-W1.
            for gi, tl in enumerate(groups):
                gcht = gglo[gi] + gghi[gi]
                hgx = wp.tile([P, GCHM, P], bf16, tag="hg_x", name="hgx")
                nc.sync.dma_start(
                    out=hgx[:, 0:gcht, :],
                    in_=xg_in[:, gch_off[gi] * P:(gch_off[gi] + gcht) * P]
                        .rearrange("p (g d) -> p g d", d=P),
                )
                maskg = group_mask(gi)
                for t in tl:
                    aggT = pp.tile([P, 264], fp32, tag="agg", space="PSUM",
                                   name="aggT")
                    chunks = []
                    for off, G in tile_ranges(gi, t):
                        chunks += list(range(off, off + G))
                    for i, g in enumerate(chunks):
                        nc.tensor.matmul(
                            out=aggT[:, 0:P],
                            lhsT=hgx[:, g, :],
                            rhs=maskg[:, g, :],
                            start=(i == 0),
                            stop=(i == len(chunks) - 1),
                        )
                    aggTs = sp.tile([P, P], bf16, tag="xaggT", name="xaggT")
                    nc.vector.tensor_copy(aggTs[:], aggT[:, 0:P])
                    ps = pp.tile([P, 264], fp32, tag="wout", space="PSUM",
                                 name="psl1")
                    nc.tensor.matmul(out=ps[:, 0:HC], lhsT=aggTs[:],
                                     rhs=w1_sb[:], start=True, stop=True)
                    pre = sp.tile([P, HC], fp32, tag="pre64", name="pre")
                    nc.scalar.activation(pre[:], ps[:, 0:HC], AF.Copy,
                                         scale=degO[:, t:t + 1])
                    nc.vector.tensor_tensor(out=pre[:], in0=pre[:],
                                            in1=b_sb["b1"][:], op=ALU.add)
                    h = elu_to_bf16(pre, HC, "64")
                    tp = pp.tile([P, P], bf16, tag="tpose", space="PSUM",
                                 name="tp")
                    nc.tensor.transpose(tp[0:HC, :], h[:], ident[:])
                    ht = sp.tile([HC, P], bf16, tag="ht64", name="ht")
                    nc.vector.tensor_copy(ht[:], tp[0:HC, :])
                    wout = pp.tile([P, 264], fp32, tag="wout", space="PSUM",
                                   name="wout")
                    nc.tensor.matmul(out=wout[:, 0:HC], lhsT=ht[:],
                                     rhs=w2_sb[:], start=True, stop=True)
                    tt = sp.tile([P, HC], bf16, tag="ttile_t2", name="ttile")
                    nc.scalar.activation(tt[:], wout[:, 0:HC], AF.Copy,
                                         scale=degO[:, t:t + 1])
                    nc.sync.dma_start(out=T2s[t * P:(t + 1) * P, 0:HC],
                                      in_=tt[:])

            # ================= GCN layers ===================================
            def gcn_layer(tbl, bias, wnext, wnext_width, slice_out, slice_width,
                          scale_next, ad_save, lname):
                for gi, tl in enumerate(groups):
                    hg = wp.tile([P, GCHM, ST_GCN], bf16, tag="hg_gcn", name="hg")
                    grouped_gather(gi, hg, ST_GCN, tbl, 64)
                    maskg = group_mask(gi)
                    for t in tl:
                        acc = agg_matmuls(gi, t, maskg, hg, HC)
                        pre = sp.tile([P, HC], fp32, tag="pre64", name="pre")
                        nc.scalar.activation(pre[:], acc[:, 0:HC], AF.Copy,
                                             scale=degO[:, t:t + 1])
                        nc.vector.tensor_tensor(out=pre[:], in0=pre[:],
                                                in1=b_sb[bias][:], op=ALU.add)
                        h = elu_to_bf16(pre, HC, "64")
                        tp = pp.tile([P, P], bf16, tag="tpose", space="PSUM", name="tp")
                        nc.tensor.transpose(tp[0:HC, :], h[:], ident[:])
                        ht = sp.tile([HC, P], bf16, tag="ht64", name="ht")
                        nc.vector.tensor_copy(ht[:], tp[0:HC, :])
                        wout = pp.tile([P, 264], fp32, tag="wout", space="PSUM", name="wout")
                        nc.tensor.matmul(out=wout[:, 0:wnext_width], lhsT=ht[:],
                                         rhs=wnext[:], start=True, stop=True)
                        tt = sp.tile([P, wnext_width], bf16, tag=f"ttile_{lname}",
                                     name="ttile")
                        if scale_next is not None:
                            nc.scalar.activation(tt[:], wout[:, 0:wnext_width], AF.Copy,
                                                 scale=scale_next[:, t:t + 1])
                        else:
                            nc.vector.tensor_copy(tt[:], wout[:, 0:wnext_width])
                        if ad_save is not None:
                            nc.vector.tensor_copy(ad_save[:, t * HEADS:(t + 1) * HEADS],
                                                  wout[:, 260:264])
                        nc.sync.dma_start(
                            out=slice_out[t * P:(t + 1) * P, 0:slice_width],
                            in_=tt[:],
                        )

            nc.gpsimd.collective_compute(
                "AllGather", mybir.AluOpType.bypass, replica_groups=RG,
                ins=[T2s[:, :]], outs=[T2[:, :]],
            )
            gcn_layer(T2, "b2", w3_sb[:], 264, T3s, 264, None, ad3, "t3")
            nc.gpsimd.collective_compute(
                "AllGather", mybir.AluOpType.bypass, replica_groups=RG,
                ins=[T3s[:, :]], outs=[T3[:, :]],
            )

            # ================= GAT layers ===================================
            def gat_attn_inplace(gi, t, hg, ad_prev, nheads, fw, hc):
                """Per-tile, per-range: hg[:, :, fw:fw+nheads] <- exp(lrelu(
                as+ad)-M); hg[:, :, 0:fw] *= that (broadcast per head)."""
                for off, G in tile_ranges(gi, t):
                    e = sp.tile([P, CHM, HEADS], fp32, tag="ebuf", name="ebuf")
                    nc.vector.tensor_tensor(
                        out=e[:, 0:G, 0:nheads],
                        in0=hg[:, off:off + G, fw:fw + nheads],
                        in1=ad_prev[:, t * nheads:(t + 1) * nheads]
                            .rearrange("p (g h) -> p g h", g=1)
                            .to_broadcast([P, G, nheads]),
                        op=ALU.add,
                    )
                    e2 = sp.tile([P, CHM, HEADS], fp32, tag="ebuf2", name="ebuf2")
                    nc.vector.tensor_scalar_mul(e2[:, 0:G, 0:nheads],
                                                e[:, 0:G, 0:nheads], NEG)
                    nc.vector.tensor_tensor(out=e2[:, 0:G, 0:nheads],
                                            in0=e2[:, 0:G, 0:nheads],
                                            in1=e[:, 0:G, 0:nheads], op=ALU.max)
                    nc.scalar.activation(hg[:, off:off + G, fw:fw + nheads],
                                         e2[:, 0:G, 0:nheads],
                                         AF.Exp, bias=mb_sb[:, 0:1])
                    nc.vector.tensor_tensor(
                        out=hg[:, off:off + G, 0:fw].rearrange(
                            "p g (h z) -> p g h z", z=hc),
                        in0=hg[:, off:off + G, 0:fw].rearrange(
                            "p g (h z) -> p g h z", z=hc),
                        in1=hg[:, off:off + G, fw:fw + nheads]
                            .rearrange("p g (h z) -> p g h z", z=1)
                            .to_broadcast([P, G, nheads, hc]),
                        op=ALU.mult,
                    )

            def gat_layer(tbl, ad_prev, bias, wnext_sb, wnext_width, slice_out,
                          slice_width, ad_save, ad_width, lname):
                for gi, tl in enumerate(groups):
                    hg = wp.tile([P, GCHM, ST_GAT], bf16, tag="hg_gat", name="hg")
                    grouped_gather(gi, hg, ST_GAT, tbl, 128)
                    maskg = group_mask(gi)
                    for t in tl:
                        gat_attn_inplace(gi, t, hg, ad_prev, HEADS, 256, HC)
                        acc = agg_matmuls(gi, t, maskg, hg, 260)
                        rs = sp.tile([P, HEADS], fp32, tag="rs4", name="rs")
                        nc.vector.tensor_scalar_max(rs[:], acc[:, 256:260], 1e-30)
                        nc.vector.reciprocal(rs[:], rs[:])
                        pre = sp.tile([P, 256], fp32, tag="pre256", name="pre")
                        nc.vector.tensor_tensor(
                            out=pre[:].rearrange("p (h z) -> p h z", z=HC),
                            in0=acc[:, 0:256].rearrange("p (h z) -> p h z", z=HC),
                            in1=rs[:].rearrange("p (h z) -> p h z", z=1)
                                .to_broadcast([P, HEADS, HC]),
                            op=ALU.mult,
                        )
                        nc.vector.tensor_tensor(out=pre[:], in0=pre[:],
                                                in1=b_sb[bias][:], op=ALU.add)
                        h = elu_to_bf16(pre, 256, "256")
                        ht = sp.tile([P, 2, P], bf16, tag="ht256", name="ht")
                        for k in range(2):
                            tp = pp.tile([P, P], bf16, tag="tpose", space="PSUM",
                                         name="tp")
                            nc.tensor.transpose(tp[:], h[:, k * P:(k + 1) * P], ident[:])
                            nc.vector.tensor_copy(ht[:, k, :], tp[:])
                        wout = pp.tile([P, 264], fp32, tag="wout", space="PSUM",
                                       name="wout")
                        for k in range(2):
                            nc.tensor.matmul(out=wout[:, 0:wnext_width],
                                             lhsT=ht[:, k, :],
                                             rhs=wnext_sb[:, k, :wnext_width],
                                             start=(k == 0), stop=(k == 1))
                        tt = sp.tile([P, wnext_width], bf16, tag=f"ttile_{lname}",
                                     name="ttile")
                        nc.vector.tensor_copy(tt[:], wout[:, 0:wnext_width])
                        nc.vector.tensor_copy(
                            ad_save[:, t * ad_width:(t + 1) * ad_width],
                            wout[:, wnext_width - ad_width:wnext_width])
                        nc.sync.dma_start(
                            out=slice_out[t * P:(t + 1) * P, 0:slice_width],
                            in_=tt[:],
                        )

            gat_layer(T3, ad3, "b3", w4_sb, 264, T4s, 264, ad4, HEADS, "t4")
            nc.gpsimd.collective_compute(
                "AllGather", mybir.AluOpType.bypass, replica_groups=RG,
                ins=[T4s[:, :]], outs=[T4[:, :]],
            )
            gat_layer(T4, ad4, "b4", w5_sb, 34, T5s, 34, ad5, 1, "t5")
            nc.gpsimd.collective_compute(
                "AllGather", mybir.AluOpType.bypass, replica_groups=RG,
                ins=[T5s[:, :]], outs=[T5[:, :]],
            )

            # ================= L5: single-head GAT + log_softmax ============
            for gi, tl in enumerate(groups):
                hg = wp.tile([P, GCHM, ST_L5], bf16, tag="hg_gcn", name="hg")
                grouped_gather(gi, hg, ST_L5, T5, 32)
                maskg = group_mask(gi)
                for t in tl:
                    gat_attn_inplace(gi, t, hg, ad5, 1, 32, 32)
                    acc = agg_matmuls(gi, t, maskg, hg, 33)
                    rs = sp.tile([P, 1], fp32, tag="rs1", name="rs")
                    nc.vector.tensor_scalar_max(rs[:], acc[:, 32:33], 1e-30)
                    nc.vector.reciprocal(rs[:], rs[:])
                    logits = sp.tile([P, OUT], fp32, tag="logits", name="logits")
                    nc.scalar.activation(logits[:], acc[:, 0:OUT], AF.Copy,
                                         scale=rs[:, 0:1])
                    nc.vector.tensor_tensor(out=logits[:], in0=logits[:],
                                            in1=b_sb["b5"][:], op=ALU.add)
                    mneg = sp.tile([P, 1], fp32, tag="mneg", name="mneg")
                    nc.vector.tensor_reduce(out=mneg[:], in_=logits[:],
                                            axis=mybir.AxisListType.X,
                                            op=ALU.max, negate=True)
                    ex2 = sp.tile([P, OUT], fp32, tag="ex2", name="ex2")
                    ssum = sp.tile([P, 1], fp32, tag="ssum", name="ssum")
                    nc.scalar.activation(ex2[:], logits[:], AF.Exp, bias=mneg[:, 0:1],
                                         accum_out=ssum[:])
                    lg = sp.tile([P, 1], fp32, tag="lg", name="lg")
                    nc.scalar.activation(lg[:], ssum[:], AF.Ln)
                    cc2 = sp.tile([P, 1], fp32, tag="cc", name="cc2")
                    nc.vector.tensor_tensor(out=cc2[:], in0=mneg[:], in1=lg[:],
                                            op=ALU.subtract)
                    outt = sp.tile([P, OUT], fp32, tag="outt", name="outt")
                    nc.scalar.activation(outt[:], logits[:], AF.Identity,
                                         bias=cc2[:, 0:1])
                    nc.sync.dma_start(out=out_ext[t * P:(t + 1) * P, :], in_=outt[:])

    nc.finalize()
    return nc


def _prepare(inputs):
    edge_index = np.asarray(inputs["edge_index"])
    plan = _build_plan(edge_index)

    x = np.asarray(inputs["x"], np.float32)
    deg = plan["deg"]
    dinv = np.where(deg > 0, 1.0 / np.sqrt(deg), 0.0).astype(np.float32)
    xs = (x * dinv[:, None]).astype(BF)          # [N, IN] src-scaled rows
    CH = plan["CH"]
    xg_maps = []
    for c in range(NCORES):
        srcm = plan["xgsrc"][c]                  # [P, CH] node id or -1
        xg = np.zeros((P, CH, IN), BF)
        valid = srcm >= 0
        xg[valid] = xs[srcm[valid]]
        xg_maps.append(np.ascontiguousarray(xg.reshape(P, CH * IN)))

    W3e = _fuse_gat_weights(inputs["gat1_W"], inputs["gat1_as"], inputs["gat1_ad"], HEADS, HC)
    W4e = _fuse_gat_weights(inputs["gat2_W"], inputs["gat2_as"], inputs["gat2_ad"], HEADS, HC)
    W5e = _fuse_gat_weights(inputs["gat3_W"], inputs["gat3_as"], inputs["gat3_ad"], 1, OUT)
    W4p = np.ascontiguousarray(
        W4e.reshape(2, P, 264).transpose(1, 0, 2).reshape(P, 2 * 264))
    W5p = np.ascontiguousarray(
        W5e.reshape(2, P, 34).transpose(1, 0, 2).reshape(P, 2 * 34))

    common = {
        "degT": plan["degT"],
        "W1": np.asarray(inputs["gcn1_W"], np.float32).astype(BF),
        "W2": np.asarray(inputs["gcn2_W"], np.float32).astype(BF),
        "W3e": W3e.astype(BF),
        "W4p": W4p.astype(BF),
        "W5p": W5p.astype(BF),
        "b1": np.asarray(inputs["gcn1_b"], np.float32).reshape(1, -1),
        "b2": np.asarray(inputs["gcn2_b"], np.float32).reshape(1, -1),
        "b3": np.asarray(inputs["gat1_b"], np.float32).reshape(1, -1),
        "b4": np.asarray(inputs["gat2_b"], np.float32).reshape(1, -1),
        "b5": np.asarray(inputs["gat3_b"], np.float32).reshape(1, -1),
    }
    in_maps = []
    for c in range(NCORES):
        m = dict(common)
        m["degO"] = np.ascontiguousarray(plan["degT"][:, c * TPC:(c + 1) * TPC])
        m["idx16"] = np.ascontiguousarray(plan["idx16"][c])
        m["dstloc"] = np.ascontiguousarray(plan["dstloc_bf"][c])
        m["xg"] = xg_maps[c]
        in_maps.append(m)
    return plan, in_maps


def kernel(**inputs) -> np.ndarray:
    plan, in_maps = _prepare(inputs)
    nc = _build_nc(plan)
    res = run_bass_kernel_spmd(nc, in_maps, core_ids=list(range(NCORES)))
    out_g = np.concatenate([res.results[c]["out"] for c in range(NCORES)], axis=0)
    return np.ascontiguousarray(out_g[plan["gsid_of"]]).astype(np.float32)


if __name__ == "__main__":
    rng = np.random.default_rng(0)
    data = dict(np.load("/tmp/inputs.npz"))
    out = kernel(**data)
    exp = np.load("/tmp/expected.npy")
    d = np.abs(out - exp)
    print("max abs:", d.max(), "rel:", d.max() / np.abs(exp).max())

